# revision 1
# baseline (speedup 1.0000x reference)
"""ArcFace loss kernel for 8 TRN2 NeuronCores — fp8 DoubleRow redesign.

Reference computation:
    w_n   = weight / max(||weight_row||, 1e-12)            # [C, D]
    cos   = emb @ w_n.T                                    # [B, C]
    logit = SCALE * cos;  logit[b, lab[b]] -= SCALE*MARGIN
    loss  = mean_b( logsumexp(logit[b]) - logit[b, lab[b]] )

Sharding: classes (C=100000) split over 8 cores (12500 each, padded to
12544), embeddings + labels replicated.

Host prep (layout/dtype/indexing only): per-core w shard is quantized to
fp8-e4m3 (pre-scaled x128 into e4m3's normal range; the factor cancels in
SCALE*r/sqrt(ssq)) and pre-transposed to [D, c_pad]; embeddings
pre-transposed/quantized to [D, B] fp8 once; the label rows w[lab] are
host-gathered (pure indexing) and passed in f32 for the exact label-logit
path, with rows not owned by the core zeroed.

Device pipeline per core, [class-partition, batch-free] layout throughout:
  - logits r[c,b] accumulate in PSUM via fp8 DoubleRow matmuls
    (lhsT = wT k-pair block, rhs = embT k-pair, 2 instrs per K=512)
  - row norms: DVE squares wT (bf16), flipped ones-matmuls (wsq stationary,
    out free = 1) partition-reduce into a PSUM column per class block;
    Newton-rsqrt on DVE converts to SCALE/||w_c|| per class
  - softmax-exp with a CONSTANT shift K (logits are ~N(0,30); the shift
    window [max_row_logit-88, min_row_logit+87] is huge, so no row-max
    pass at all): ACT computes E = exp(inv[c]*r - K) with the per-class
    norm applied via the per-partition activation scale
  - sum over classes: flipped ones-matmuls (E slice stationary) accumulate
    S[128b, btile] in a dedicated PSUM bank across all class blocks
    (single start=True on the bank's first matmul)
  - label logits T from the host-gathered f32 rows + DVE dot/ssq
  - one AllGather of (S_k, T_k) [128, 16]; every core combines and
    computes loss_b = K + ln(S - e^{T-K} + e^{T-K-SM}) - T + SM, then the
    batch mean via a ones-matmul; core 0's scalar is returned.
"""

import numpy as np
from contextlib import ExitStack

B = 1024
D = 512
C = 100000
NCORES = 8
C_LOC = C // NCORES          # 12500
C_PAD = ((C_LOC + 127) // 128) * 128   # 12544
NBLK = C_PAD // 128          # 98
ND = D // 128                # 4
NB = B // 128                # 8
SCALE = 30.0
MARGIN = 0.5
SM = SCALE * MARGIN          # 15.0
K_SHIFT = 150.0              # constant softmax shift (see module docstring)

# class blocks per pipeline chunk (first chunk small to prime the
# exp pipeline quickly)
CHUNKS = [4, 8, 12, 16, 16, 16, 16, 10]
assert sum(CHUNKS) == NBLK

RSQRT_MAGIC = 0x5F3759DF


def _dve_rsqrt_scale(nc, pool, x, n, scale_out, bufs=2, out=None):
    """out = scale_out / sqrt(x) elementwise on a [128, n] tile (DVE only:
    quake initial guess + 2 Newton steps; ACT Rsqrt is banned)."""
    import concourse.mybir as mybir
    f32 = mybir.dt.float32
    i32 = mybir.dt.int32
    y = pool.tile([128, n], f32, tag="rsq_y", bufs=bufs, name="rsq_y")
    t = pool.tile([128, n], f32, tag="rsq_t", bufs=bufs, name="rsq_t")
    xc = pool.tile([128, n], f32, tag="rsq_x", bufs=bufs, name="rsq_x")
    nc.vector.tensor_scalar(
        out=xc, in0=x, scalar1=1e-12, scalar2=None, op0=mybir.AluOpType.max)
    x = xc
    xi = x.bitcast(i32)
    yi = y.bitcast(i32)
    nc.vector.tensor_scalar(
        out=yi, in0=xi, scalar1=1, scalar2=-1,
        op0=mybir.AluOpType.arith_shift_right, op1=mybir.AluOpType.bitwise_xor)
    nc.vector.tensor_scalar(
        out=yi, in0=yi, scalar1=RSQRT_MAGIC + 1, scalar2=None,
        op0=mybir.AluOpType.add)
    for it in range(2):
        nc.vector.tensor_tensor(out=t, in0=y, in1=y, op=mybir.AluOpType.mult)
        nc.vector.tensor_tensor(out=t, in0=t, in1=x, op=mybir.AluOpType.mult)
        nc.vector.tensor_scalar(
            out=t, in0=t, scalar1=-0.5, scalar2=1.5,
            op0=mybir.AluOpType.mult, op1=mybir.AluOpType.add)
        if it == 1:
            nc.vector.tensor_scalar(
                out=t, in0=t, scalar1=float(scale_out), scalar2=None,
                op0=mybir.AluOpType.mult)
        dst = y if (it == 0 or out is None) else out
        nc.vector.tensor_tensor(out=dst, in0=y, in1=t, op=mybir.AluOpType.mult)
    return out if out is not None else y


def build_nc(n_cores=NCORES, debug_outs=False):
    import concourse.bass as bass
    import concourse.tile as tile
    import concourse.mybir as mybir
    from concourse import bacc

    f32 = mybir.dt.float32
    bf16 = mybir.dt.bfloat16
    f8 = mybir.dt.float8e4
    i32 = mybir.dt.int32
    Alu = mybir.AluOpType
    Act = mybir.ActivationFunctionType
    DR = mybir.MatmulPerfMode.DoubleRow

    nc = bacc.Bacc()

    # fp8 transposed operands (row index d = ks*128 + p)
    wt8 = nc.declare_dram_parameter("wt8", [D, C_PAD], f8, isOutput=False)
    et8 = nc.declare_dram_parameter("et8", [D, B], f8, isOutput=False)
    # f32 label rows (host-gathered, zeroed when not owned) + embeddings
    # for the exact label-logit path
    wlab = nc.declare_dram_parameter("wlab", [B, D], f32, isOutput=False)
    emb = nc.declare_dram_parameter("emb", [B, D], f32, isOutput=False)
    lab = nc.declare_dram_parameter("lab", [B], i32, isOutput=False)
    out = nc.declare_dram_parameter("out", [1], f32, isOutput=True)
    if debug_outs:
        s_dbg = nc.declare_dram_parameter("s_dbg", [128, 16], f32, isOutput=True)
        inv_dbg = nc.declare_dram_parameter("inv_dbg", [128, NBLK], f32, isOutput=True)
        st_dbg = nc.declare_dram_parameter("st_dbg", [128, 16], f32, isOutput=True)
        lp_dbg = nc.declare_dram_parameter("lp_dbg", [128, 4 * NB], f32, isOutput=True)
        wg_dbg = nc.declare_dram_parameter("wg_dbg", [128, NB, D], f32, isOutput=True)
        tl_dbg = nc.declare_dram_parameter("tl_dbg", [128, 5 * NB + 1], f32, isOutput=True)

    with ExitStack() as ctx:
        tc = ctx.enter_context(tile.TileContext(nc))
        dram = ctx.enter_context(tc.tile_pool(name="dram", bufs=1, space="DRAM"))
        res = ctx.enter_context(tc.tile_pool(name="res", bufs=1))
        work = ctx.enter_context(tc.tile_pool(name="work", bufs=2))
        psum = ctx.enter_context(tc.tile_pool(name="psum", bufs=1, space="PSUM"))

        # collective bounce buffers
        st_in = dram.tile([128, 16], f32, tag="st_in", name="st_in")
        st_out = dram.tile([n_cores * 128, 16], f32, tag="st_out",
                           name="st_out", addr_space="Shared")

        ones = res.tile([128, 1], bf16, tag="ones", name="ones")
        nc.vector.memset(ones, 1.0)
        kbias = res.tile([128, 1], f32, tag="kbias", name="kbias")
        nc.vector.memset(kbias, -K_SHIFT)
        kbias2 = res.tile([128, 1], f32, tag="kbias2", name="kbias2")
        nc.vector.memset(kbias2, -(K_SHIFT + SM))
        # dummy activation so the ACT table load lands here, before the DVE
        # norm stream exists to be (coarsely) waited on
        warm = res.tile([128, 1], f32, tag="warm", name="warm")
        nc.scalar.activation(out=warm, in_=kbias[:, 0:1], func=Act.Exp)

        # ---------------- loads ----------------
        # first weight chunk loads before the (larger) embedding operand:
        # the norm chain for chunk 0 gates the whole exp stream
        wt_first = res.tile([128, ND, CHUNKS[0] * 128], f8, tag="wt0",
                            name="wt0")
        nc.sync.dma_start(
            out=wt_first,
            in_=wt8[:, 0:CHUNKS[0] * 128].rearrange("(k p) c -> p k c", p=128))
        et = res.tile([128, ND, B], f8, tag="et", name="et")
        nc.sync.dma_start(out=et, in_=et8.rearrange("(k p) b -> p k b", p=128))

        S_col = psum.tile([128, 512], f32, tag="S", name="S", space="PSUM")

        # ---------------- main pipeline over class chunks ----------------
        blk0 = 0
        first_s = [True]
        for ci, cb in enumerate(CHUNKS):
            c0, c1 = blk0 * 128, (blk0 + cb) * 128
            cw = c1 - c0
            # per-chunk tiles so dependencies stay chunk-local
            if ci == 0:
                wt = wt_first
            else:
                wt = res.tile([128, ND, cw], f8, tag=f"wt{ci}", name=f"wt{ci}")
                nc.sync.dma_start(
                    out=wt,
                    in_=wt8[:, c0:c1].rearrange("(k p) c -> p k c", p=128))
            # norms: DVE squares -> flipped PE reduce -> DVE rsqrt.
            # high_priority keeps this chain ahead of bulkier DVE work in
            # the Tile scheduler's per-engine ready heaps — the exp stream
            # gates on inv_c, so norm production must never queue behind
            # later chunks' squares.
            inv_c = res.tile([128, cb], f32, tag=f"inv{ci}", name=f"inv{ci}")
            if True:
                wsq = work.tile([128, ND, 2048], bf16, tag="wsq", bufs=2,
                                name="wsq")
                # per-block pieces: the serial rsqrt chain has idle gaps the
                # greedy per-engine scheduler fills with whatever is ready —
                # small squares bound that gap-filling to ~0.5us per step
                # (a single chunk-wide square would add 8.5us per gap)
                for bl in range(cb):
                    nc.vector.tensor_tensor(
                        out=wsq[:, :, bl * 128:(bl + 1) * 128],
                        in0=wt[:, :, bl * 128:(bl + 1) * 128],
                        in1=wt[:, :, bl * 128:(bl + 1) * 128],
                        op=Alu.mult)
                sq_col = psum.tile([128, 512], f32, tag="sq", bufs=2, name="sq",
                                   space="PSUM")
                for bl in range(cb):
                    for k in range(ND):
                        nc.tensor.matmul(
                            sq_col[:, bl:bl + 1],
                            wsq[:, k, bl * 128:(bl + 1) * 128], ones,
                            start=(bl == 0 and k == 0),
                            stop=(bl == cb - 1 and k == ND - 1),
                            skip_group_check=True)
                _dve_rsqrt_scale(nc, work, sq_col[:, 0:cb], cb, SCALE,
                                 out=inv_c)

            # logits + exp + flipped sum-exp
            for bl in range(cb):
                blk = blk0 + bl
                pt = psum.tile([128, B], f32, tag="pt", bufs=2, name="pt",
                               space="PSUM")
                for h in range(2):
                    for kp in range(ND // 2):
                        nc.tensor.matmul(
                            pt[:, h * 512:(h + 1) * 512],
                            wt[:, 2 * kp:2 * kp + 2,
                               bl * 128:(bl + 1) * 128],
                            et[:, 2 * kp:2 * kp + 2, h * 512:(h + 1) * 512],
                            start=(kp == 0), stop=(kp == ND // 2 - 1),
                            perf_mode=DR)
                E = work.tile([128, B], bf16, tag="E", bufs=3, name="E")
                nc.scalar.activation(
                    out=E, in_=pt, func=Act.Exp,
                    bias=kbias[:, 0:1], scale=inv_c[:, bl:bl + 1])
                for t in range(NB):
                    nc.tensor.matmul(
                        S_col[:, t:t + 1],
                        E[:, t * 128:(t + 1) * 128], ones,
                        start=first_s[0],
                        stop=(blk == NBLK - 1 and t == NB - 1),
                        skip_group_check=True)
                    first_s[0] = False
            blk0 += cb

        # ---------------- label-logit path (exact, f32) ----------------
        e_all = res.tile([128, NB, D], f32, tag="e_all", name="e_all")
        nc.sync.dma_start(out=e_all, in_=emb.rearrange("(t p) d -> p t d", p=128))
        lab_s = res.tile([128, NB], i32, tag="lab_s", name="lab_s")
        nc.sync.dma_start(out=lab_s, in_=lab.rearrange("(t p) -> p t", p=128))
        labf = res.tile([128, NB], f32, tag="labf", name="labf")
        nc.vector.tensor_copy(out=labf, in_=lab_s)
        wg_all = res.tile([128, NB, D], f32, tag="wg_all", name="wg_all")
        nc.sync.dma_start(
            out=wg_all, in_=wlab.rearrange("(t p) d -> p t d", p=128))
        ldot = res.tile([128, NB], f32, tag="ldot", name="ldot")
        ssg = res.tile([128, NB], f32, tag="ssg", name="ssg")
        prod = work.tile([128, NB, D], f32, tag="prod", bufs=1, name="prod")
        nc.vector.tensor_tensor(out=prod, in0=wg_all, in1=e_all, op=Alu.mult)
        for t in range(NB):
            nc.vector.tensor_reduce(
                out=ldot[:, t:t + 1], in_=prod[:, t, :],
                axis=mybir.AxisListType.X, op=Alu.add)
        sqg = work.tile([128, NB, D], f32, tag="sqg", bufs=1, name="sqg")
        nc.vector.tensor_tensor(out=sqg, in0=wg_all, in1=wg_all, op=Alu.mult)
        for t in range(NB):
            nc.vector.tensor_reduce(
                out=ssg[:, t:t + 1], in_=sqg[:, t, :],
                axis=mybir.AxisListType.X, op=Alu.add)
        linv = _dve_rsqrt_scale(nc, res, ssg, NB, SCALE)
        T_st = res.tile([128, NB], f32, tag="T_st", name="T_st")
        nc.vector.tensor_tensor(out=T_st, in0=ldot, in1=linv, op=Alu.mult)
        # zero out labels owned by other cores
        msk = res.tile([128, NB], f32, tag="msk", name="msk")
        nc.vector.tensor_scalar(
            out=msk, in0=labf, scalar1=0.0, scalar2=None, op0=Alu.is_ge)
        nc.vector.tensor_tensor(out=T_st, in0=T_st, in1=msk, op=Alu.mult)
        nc.vector.tensor_scalar(
            out=msk, in0=labf, scalar1=float(C_LOC - 1), scalar2=None,
            op0=Alu.is_le)
        nc.vector.tensor_tensor(out=T_st, in0=T_st, in1=msk, op=Alu.mult)

        # ---------------- collective: AllGather (S_k, T_k) ----------------
        stpack = res.tile([128, 16], f32, tag="stpack", name="stpack")
        nc.vector.tensor_copy(out=stpack[:, 0:NB], in_=S_col[:, 0:NB])
        nc.vector.tensor_copy(out=stpack[:, NB:16], in_=T_st)
        nc.sync.dma_start(out=st_in[:, :], in_=stpack)
        nc.gpsimd.collective_compute(
            "AllGather", Alu.bypass,
            replica_groups=[list(range(n_cores))],
            ins=[st_in[:, :]], outs=[st_out[:, :]])
        AG = res.tile([128, n_cores, 16], f32, tag="AG", name="AG")
        nc.sync.dma_start(
            out=AG, in_=st_out[:, :].rearrange("(kk p) c -> p kk c", p=128))
        ST = res.tile([128, 16], f32, tag="STg", name="STg")
        nc.vector.tensor_reduce(
            out=ST, in_=AG.rearrange("p k c -> p c k"),
            axis=mybir.AxisListType.X, op=Alu.add)
        SG = ST[:, 0:NB]
        TG = ST[:, NB:16]

        # loss_b = K + ln(SG - e^{T-K} + e^{T-K-SM}) - T + SM
        ea = res.tile([128, NB], f32, tag="ea", name="ea")
        nc.scalar.activation(out=ea, in_=TG, func=Act.Exp, bias=kbias[:, 0:1])
        eb = res.tile([128, NB], f32, tag="eb", name="eb")
        nc.scalar.activation(out=eb, in_=TG, func=Act.Exp, bias=kbias2[:, 0:1])
        S2 = res.tile([128, NB], f32, tag="S2", name="S2")
        nc.vector.tensor_tensor(out=S2, in0=SG, in1=ea, op=Alu.subtract)
        nc.vector.tensor_tensor(out=S2, in0=S2, in1=eb, op=Alu.add)
        # ln(S2) with S2 spanning e^-25..e^60: the ACT Ln table misbehaves on
        # small inputs, so split off the exponent on DVE and Ln only the
        # mantissa in [1, 2)
        xi = S2.bitcast(i32)
        ei = res.tile([128, NB], i32, tag="ei", name="ei")
        nc.vector.tensor_scalar(
            out=ei, in0=xi, scalar1=23, scalar2=None,
            op0=Alu.logical_shift_right)
        nc.vector.tensor_scalar(
            out=ei, in0=ei, scalar1=-127, scalar2=None, op0=Alu.add)
        ef = res.tile([128, NB], f32, tag="ef", name="ef")
        nc.vector.tensor_copy(out=ef, in_=ei)
        mb = res.tile([128, NB], i32, tag="mb", name="mb")
        nc.vector.tensor_scalar(
            out=mb, in0=xi, scalar1=0x007FFFFF, scalar2=0x3F800000,
            op0=Alu.bitwise_and, op1=Alu.bitwise_or)
        lg = res.tile([128, NB], f32, tag="lg", name="lg")
        nc.scalar.activation(out=lg, in_=mb.bitcast(f32), func=Act.Ln)
        lg2 = res.tile([128, NB], f32, tag="lg2", name="lg2")
        nc.vector.tensor_scalar(
            out=lg2, in0=ef, scalar1=float(np.log(2.0)), scalar2=None,
            op0=Alu.mult)
        nc.vector.tensor_tensor(out=lg2, in0=lg2, in1=lg, op=Alu.add)
        nc.vector.tensor_tensor(out=lg2, in0=lg2, in1=TG, op=Alu.subtract)
        nc.vector.tensor_scalar(
            out=lg2, in0=lg2, scalar1=K_SHIFT + SM, scalar2=None, op0=Alu.add)

        if debug_outs:
            nc.sync.dma_start(out=s_dbg[:, :], in_=stpack)
            nc.sync.dma_start(out=inv_dbg[:, :], in_=inv_all)
            nc.sync.dma_start(out=st_dbg[:, :], in_=ST)
            lp = res.tile([128, 4 * NB], f32, tag="lp", name="lp")
            nc.vector.tensor_copy(out=lp[:, 0:NB], in_=ldot)
            nc.vector.tensor_copy(out=lp[:, NB:2 * NB], in_=ssg)
            nc.vector.tensor_copy(out=lp[:, 2 * NB:3 * NB], in_=linv)
            nc.vector.tensor_copy(out=lp[:, 3 * NB:4 * NB], in_=labf)
            nc.sync.dma_start(out=lp_dbg[:, :], in_=lp)
            nc.sync.dma_start(out=wg_dbg[:, :, :], in_=wg_all)

        rs = res.tile([128, 1], f32, tag="rs", name="rs")
        nc.vector.tensor_reduce(
            out=rs, in_=lg2, axis=mybir.AxisListType.X, op=Alu.add)
        ones_f = res.tile([128, 1], f32, tag="ones_f", name="ones_f")
        nc.vector.memset(ones_f, 1.0)
        # final mean lands in the (long finished) S bank's spare columns
        nc.tensor.matmul(S_col[0:1, 64:65], rs, ones_f,
                         start=True, stop=True, skip_group_check=True)
        out_sb = res.tile([1, 1], f32, tag="out_sb", name="out_sb")
        nc.vector.tensor_scalar(
            out=out_sb, in0=S_col[0:1, 64:65], scalar1=1.0 / B, scalar2=None,
            op0=Alu.mult)
        nc.sync.dma_start(out=out[0:1], in_=out_sb[0:1, 0])
        if debug_outs:
            tl = res.tile([128, 5 * NB + 1], f32, tag="tl", name="tl")
            nc.vector.tensor_copy(out=tl[:, 0:NB], in_=ea)
            nc.vector.tensor_copy(out=tl[:, NB:2 * NB], in_=eb)
            nc.vector.tensor_copy(out=tl[:, 2 * NB:3 * NB], in_=S2)
            nc.vector.tensor_copy(out=tl[:, 3 * NB:4 * NB], in_=lg)
            nc.vector.tensor_copy(out=tl[:, 4 * NB:5 * NB], in_=lg2)
            nc.vector.tensor_copy(out=tl[:, 5 * NB:5 * NB + 1], in_=rs)
            nc.sync.dma_start(out=tl_dbg[:, :], in_=tl)

    nc.compile()
    return nc


def kernel(embeddings, labels, weight):
    import ml_dtypes
    import concourse.bass_utils as bass_utils

    emb = np.ascontiguousarray(np.asarray(embeddings, dtype=np.float32))
    labv = np.asarray(labels).astype(np.int64)
    w = np.asarray(weight, dtype=np.float32)

    et8 = np.ascontiguousarray(emb.T.astype(ml_dtypes.float8_e4m3))

    nc = build_nc()
    in_maps = []
    for k in range(NCORES):
        wpad = np.zeros((C_PAD, D), dtype=np.float32)
        wpad[:C_LOC] = w[k * C_LOC:(k + 1) * C_LOC]
        # pre-scale into e4m3's normal range (raw w ~ +-0.0077 would be
        # subnormal); the factor cancels in SCALE * r / sqrt(ssq)
        wt8 = np.ascontiguousarray((wpad.T * 128.0).astype(ml_dtypes.float8_e4m3))
        loc = labv - k * C_LOC
        owned = (loc >= 0) & (loc < C_LOC)
        wlab = np.where(owned[:, None], w[np.clip(labv, 0, C - 1)], 0.0).astype(np.float32)
        in_maps.append({
            "wt8": wt8,
            "et8": et8,
            "wlab": np.ascontiguousarray(wlab),
            "emb": emb,
            "lab": (labv - k * C_LOC).astype(np.int32),
        })
    res = bass_utils.run_bass_kernel_spmd(nc, in_maps, core_ids=list(range(NCORES)))
    return np.float32(np.asarray(res.results[0]["out"]).ravel()[0])

